# revision 4
# baseline (speedup 1.0000x reference)
"""JPEGBase (nn_JPEGBase_240518169043) Trainium2 kernel.

The reference computes rgb->yuv, *255, blockwise 8x8 DCT, blockwise IDCT
(compress() is identity), /255, yuv->rgb.  The orthonormal DCT/IDCT pair and
the *255 / /255 cancel exactly, so the remaining math is a per-pixel 3x3
color-matrix roundtrip A = yuv2rgb @ rgb2yuv applied along the channel dim
(float32 discrepancy vs. the reference's explicit DCT roundtrip is ~1.5e-7
relative).  i_co is unused by the reference.

Sharding: pure data parallelism - batch 32 -> 4 images per core across 8
cores.  Per core the kernel streams 4 images of [3,512,512] f32 through SBUF
([128,2048] per plane), computes the three output planes as weighted sums of
the three input planes (2 DVE scalar_tensor_tensor ops + 1 ACT scale per
output plane), and streams them back.  Memory-bound: ~25 MB of HBM traffic
per core.
"""

import numpy as np
from contextlib import ExitStack

import concourse.bass as bass  # noqa: F401  (engine namespaces live on nc)
import concourse.tile as tile
from concourse import bacc, mybir
from concourse.bass_utils import run_bass_kernel_spmd

N_CORES = 8
B_FULL = 32
B_PER_CORE = B_FULL // N_CORES  # 4
C = 3
H = 512
W = 512
P = 128               # SBUF partitions
F = (H * W) // P      # 2048 floats per partition per plane


def _color_matrix():
    # kornia rgb_to_yuv / yuv_to_rgb coefficient matrices, composed in f64.
    m = np.array(
        [[0.299, 0.587, 0.114],
         [-0.147, -0.289, 0.436],
         [0.615, -0.515, -0.100]], dtype=np.float64)
    n = np.array(
        [[1.0, 0.0, 1.14],
         [1.0, -0.396, -0.581],
         [1.0, 2.029, 0.0]], dtype=np.float64)
    return n @ m


def build_nc():
    """Build + compile the per-core Bass program (same program on all cores)."""
    a = _color_matrix()
    nc = bacc.Bacc(
        "TRN2", target_bir_lowering=False, debug=False, num_devices=N_CORES
    )
    x = nc.dram_tensor(
        "x", [B_PER_CORE, C, H, W], mybir.dt.float32, kind="ExternalInput"
    ).ap()
    y = nc.dram_tensor(
        "y", [B_PER_CORE, C, H, W], mybir.dt.float32, kind="ExternalOutput"
    ).ap()
    # [b, c, 128, 2048]; partition p covers image rows [4p, 4p+4) (contiguous)
    xr = x.rearrange("b c (hp hs) w -> b c hp (hs w)", hp=P)
    yr = y.rearrange("b c (hp hs) w -> b c hp (hs w)", hp=P)

    f32 = mybir.dt.float32
    with tile.TileContext(nc) as tc, ExitStack() as ctx:
        in_pool = ctx.enter_context(tc.tile_pool(name="in", bufs=3))
        out_pool = ctx.enter_context(tc.tile_pool(name="out", bufs=3))
        t_pool = ctx.enter_context(tc.tile_pool(name="tmp", bufs=3))

        for b in range(B_PER_CORE):
            # One 3 MB DMA per image; alternate the two HWDGE rings (SP/ACT).
            ld_eng = nc.sync if b % 2 == 0 else nc.scalar
            st_eng = nc.scalar if b % 2 == 0 else nc.sync
            it = in_pool.tile([P, C * F], f32)
            ld_eng.dma_start(
                it[:].rearrange("p (c f) -> p c f", c=C), xr[b]
            )
            ot = out_pool.tile([P, C * F], f32)
            for c in range(C):
                # out_c = a[c,i]*X_i + a[c,j]*X_j + a[c,c]*X_c with the
                # diagonal term largest; (i, j) = off-diagonals, |a_i|<=|a_j|:
                #   ot_c  = X_c * a[c,c]                    (ACT)
                #   t1    = X_i * (a[c,i]/a[c,j]) + X_j     (DVE / GpSimd stt)
                #   ot_c  = t1 * a[c,j] + ot_c              (DVE stt, in-place)
                i, j = [d for d in range(C) if d != c]
                if abs(a[c, i]) > abs(a[c, j]):
                    i, j = j, i
                sl = lambda d: slice(d * F, (d + 1) * F)
                nc.scalar.mul(ot[:, sl(c)], it[:, sl(c)], float(a[c, c]))
                t1 = t_pool.tile([P, F], f32)
                if (b * C + c) % 3 == 0:
                    # Offload to GpSimd: Pool rejects TensorScalarPtr, so ACT
                    # pre-scales (single-src, cheap) and Pool does a plain add.
                    xs = t_pool.tile([P, F], f32, tag="xs")
                    nc.scalar.mul(xs[:], it[:, sl(i)], float(a[c, i] / a[c, j]))
                    nc.gpsimd.tensor_tensor(
                        t1[:], xs[:], it[:, sl(j)], mybir.AluOpType.add
                    )
                else:
                    nc.vector.scalar_tensor_tensor(
                        t1[:], it[:, sl(i)], float(a[c, i] / a[c, j]),
                        it[:, sl(j)],
                        mybir.AluOpType.mult, mybir.AluOpType.add,
                    )
                nc.vector.scalar_tensor_tensor(
                    ot[:, sl(c)], t1[:], float(a[c, j]), ot[:, sl(c)],
                    mybir.AluOpType.mult, mybir.AluOpType.add,
                )
            st_eng.dma_start(
                yr[b], ot[:].rearrange("p (c f) -> p c f", c=C)
            )

    nc.compile()
    return nc


_NC = None


def _get_nc():
    global _NC
    if _NC is None:
        _NC = build_nc()
    return _NC


def _in_maps(i_en):
    xs = np.ascontiguousarray(np.asarray(i_en, dtype=np.float32)).reshape(
        N_CORES, B_PER_CORE, C, H, W
    )
    return [{"x": xs[i]} for i in range(N_CORES)]


def kernel(i_co=None, i_en=None, **_):
    res = run_bass_kernel_spmd(_get_nc(), _in_maps(i_en), list(range(N_CORES)))
    return np.concatenate(
        [res.results[i]["y"] for i in range(N_CORES)], axis=0
    )


# revision 5
# speedup vs baseline: 1.1270x; 1.1270x over previous
"""JPEGBase (nn_JPEGBase_240518169043) Trainium2 kernel.

The reference computes rgb->yuv, *255, blockwise 8x8 DCT, blockwise IDCT
(compress() is identity), /255, yuv->rgb.  The orthonormal DCT/IDCT pair and
the *255 / /255 cancel exactly, so the remaining math is a per-pixel 3x3
color-matrix roundtrip A = yuv2rgb @ rgb2yuv applied along the channel dim
(float32 discrepancy vs. the reference's explicit DCT roundtrip is ~1.5e-7
relative).  i_co is unused by the reference.

Sharding: pure data parallelism - batch 32 -> 4 images per core across 8
cores.  Per core the kernel streams 4 images of [3,512,512] f32 through SBUF
([128,2048] per plane), computes the three output planes as weighted sums of
the three input planes (2 DVE scalar_tensor_tensor ops + 1 ACT scale per
output plane), and streams them back.  Memory-bound: ~25 MB of HBM traffic
per core.
"""

import numpy as np
from contextlib import ExitStack

import concourse.bass as bass  # noqa: F401  (engine namespaces live on nc)
import concourse.tile as tile
from concourse import bacc, mybir
from concourse.bass_utils import run_bass_kernel_spmd

N_CORES = 8
B_FULL = 32
B_PER_CORE = B_FULL // N_CORES  # 4
C = 3
H = 512
W = 512
P = 128               # SBUF partitions
F = (H * W) // P      # 2048 floats per partition per plane


def _color_matrix():
    # kornia rgb_to_yuv / yuv_to_rgb coefficient matrices, composed in f64.
    m = np.array(
        [[0.299, 0.587, 0.114],
         [-0.147, -0.289, 0.436],
         [0.615, -0.515, -0.100]], dtype=np.float64)
    n = np.array(
        [[1.0, 0.0, 1.14],
         [1.0, -0.396, -0.581],
         [1.0, 2.029, 0.0]], dtype=np.float64)
    return n @ m


def build_nc():
    """Build + compile the per-core Bass program (same program on all cores)."""
    a = _color_matrix()
    nc = bacc.Bacc(
        "TRN2", target_bir_lowering=False, debug=False, num_devices=N_CORES
    )
    x = nc.dram_tensor(
        "x", [B_PER_CORE, C, H, W], mybir.dt.float32, kind="ExternalInput"
    ).ap()
    y = nc.dram_tensor(
        "y", [B_PER_CORE, C, H, W], mybir.dt.float32, kind="ExternalOutput"
    ).ap()
    # [b, c, 128, 2048]; partition p covers image rows [4p, 4p+4) (contiguous)
    xr = x.rearrange("b c (hp hs) w -> b c hp (hs w)", hp=P)
    yr = y.rearrange("b c (hp hs) w -> b c hp (hs w)", hp=P)

    f32 = mybir.dt.float32
    with tile.TileContext(nc) as tc, ExitStack() as ctx:
        in_pool = ctx.enter_context(tc.tile_pool(name="in", bufs=3))
        out_pool = ctx.enter_context(tc.tile_pool(name="out", bufs=2))
        t_pool = ctx.enter_context(tc.tile_pool(name="tmp", bufs=3))

        for b in range(B_PER_CORE):
            # 3 MB/image transfers (~400 GB/s vs ~340 for 1 MB).  Loads on the
            # SP HWDGE ring, stores on the ACT ring: each ring is FIFO per
            # issuing engine, so stores waiting on compute must not block
            # loads.  ACT computes the *final* op per plane, so its store
            # push never waits on another engine.
            it = in_pool.tile([P, C * F], f32)
            nc.sync.dma_start(it[:].rearrange("p (c f) -> p c f", c=C), xr[b])
            ot = out_pool.tile([P, C * F], f32)
            for c in range(C):
                # out_c = a[c,i]*X_i + a[c,j]*X_j + a[c,c]*X_c, diagonal term
                # largest; (i, j) = off-diagonals with |a_i| <= |a_j|:
                #   t1    = X_i * (a[c,i]/a[c,j]) + X_j     (DVE stt)
                #   t2    = t1 * (a[c,j]/a[c,c]) + X_c      (DVE stt)
                #   out_c = t2 * a[c,c]                     (ACT, single-src)
                i, j = [d for d in range(C) if d != c]
                if abs(a[c, i]) > abs(a[c, j]):
                    i, j = j, i
                sl = lambda d: slice(d * F, (d + 1) * F)
                t1 = t_pool.tile([P, F], f32)
                nc.vector.scalar_tensor_tensor(
                    t1[:], it[:, sl(i)], float(a[c, i] / a[c, j]), it[:, sl(j)],
                    mybir.AluOpType.mult, mybir.AluOpType.add,
                )
                t2 = t_pool.tile([P, F], f32, tag="t2")
                nc.vector.scalar_tensor_tensor(
                    t2[:], t1[:], float(a[c, j] / a[c, c]), it[:, sl(c)],
                    mybir.AluOpType.mult, mybir.AluOpType.add,
                )
                nc.scalar.mul(ot[:, sl(c)], t2[:], float(a[c, c]))
            nc.scalar.dma_start(yr[b], ot[:].rearrange("p (c f) -> p c f", c=C))

    nc.compile()
    return nc


_NC = None


def _get_nc():
    global _NC
    if _NC is None:
        _NC = build_nc()
    return _NC


def _in_maps(i_en):
    xs = np.ascontiguousarray(np.asarray(i_en, dtype=np.float32)).reshape(
        N_CORES, B_PER_CORE, C, H, W
    )
    return [{"x": xs[i]} for i in range(N_CORES)]


def kernel(i_co=None, i_en=None, **_):
    res = run_bass_kernel_spmd(_get_nc(), _in_maps(i_en), list(range(N_CORES)))
    return np.concatenate(
        [res.results[i]["y"] for i in range(N_CORES)], axis=0
    )


# revision 6
# speedup vs baseline: 1.1966x; 1.0618x over previous
"""JPEGBase (nn_JPEGBase_240518169043) Trainium2 kernel.

The reference computes rgb->yuv, *255, blockwise 8x8 DCT, blockwise IDCT
(compress() is identity), /255, yuv->rgb.  The orthonormal DCT/IDCT pair and
the *255 / /255 cancel exactly, so the remaining math is a per-pixel 3x3
color-matrix roundtrip A = yuv2rgb @ rgb2yuv applied along the channel dim
(float32 discrepancy vs. the reference's explicit DCT roundtrip is ~1.5e-7
relative).  i_co is unused by the reference.

Sharding: pure data parallelism - batch 32 -> 4 images per core across 8
cores.  Per core the kernel streams 4 images of [3,512,512] f32 through SBUF
([128,2048] per plane), computes the three output planes as weighted sums of
the three input planes (2 DVE scalar_tensor_tensor ops + 1 ACT scale per
output plane), and streams them back.  Memory-bound: ~25 MB of HBM traffic
per core.
"""

import numpy as np
from contextlib import ExitStack

import concourse.bass as bass  # noqa: F401  (engine namespaces live on nc)
import concourse.tile as tile
from concourse import bacc, mybir
from concourse.bass_utils import run_bass_kernel_spmd

N_CORES = 8
B_FULL = 32
B_PER_CORE = B_FULL // N_CORES  # 4
C = 3
H = 512
W = 512
P = 128               # SBUF partitions
F = (H * W) // P      # 2048 floats per partition per plane


def _color_matrix():
    # kornia rgb_to_yuv / yuv_to_rgb coefficient matrices, composed in f64.
    m = np.array(
        [[0.299, 0.587, 0.114],
         [-0.147, -0.289, 0.436],
         [0.615, -0.515, -0.100]], dtype=np.float64)
    n = np.array(
        [[1.0, 0.0, 1.14],
         [1.0, -0.396, -0.581],
         [1.0, 2.029, 0.0]], dtype=np.float64)
    return n @ m


def build_nc():
    """Build + compile the per-core Bass program (same program on all cores)."""
    a = _color_matrix()
    nc = bacc.Bacc(
        "TRN2", target_bir_lowering=False, debug=False, num_devices=N_CORES
    )
    x = nc.dram_tensor(
        "x", [B_PER_CORE, C, H, W], mybir.dt.float32, kind="ExternalInput"
    ).ap()
    y = nc.dram_tensor(
        "y", [B_PER_CORE, C, H, W], mybir.dt.float32, kind="ExternalOutput"
    ).ap()
    # [b, 128, c, 2048]; partition p covers image rows [4p, 4p+4) (contiguous);
    # dim order matches the SBUF tile view [p, c, f].
    xr = x.rearrange("b c (hp hs) w -> b hp c (hs w)", hp=P)
    yr = y.rearrange("b c (hp hs) w -> b hp c (hs w)", hp=P)

    f32 = mybir.dt.float32
    with tile.TileContext(nc) as tc, ExitStack() as ctx:
        in_pool = ctx.enter_context(tc.tile_pool(name="in", bufs=3))
        out_pool = ctx.enter_context(tc.tile_pool(name="out", bufs=2))
        t_pool = ctx.enter_context(tc.tile_pool(name="tmp", bufs=3))

        for b in range(B_PER_CORE):
            # 3 MB/image transfers (~400 GB/s vs ~340 for 1 MB).  Loads on the
            # SP HWDGE ring, stores on the ACT ring: each ring is FIFO per
            # issuing engine, so stores waiting on compute must not block
            # loads.  ACT computes the *final* op per plane, so its store
            # push never waits on another engine.
            it = in_pool.tile([P, C * F], f32)
            nc.sync.dma_start(it[:].rearrange("p (c f) -> p c f", c=C), xr[b])
            ot = out_pool.tile([P, C * F], f32)
            for c in range(C):
                # out_c = a[c,i]*X_i + a[c,j]*X_j + a[c,c]*X_c, diagonal term
                # largest; (i, j) = off-diagonals with |a_i| <= |a_j|:
                #   t1    = X_i * (a[c,i]/a[c,j]) + X_j     (DVE stt)
                #   t2    = t1 * (a[c,j]/a[c,c]) + X_c      (DVE stt)
                #   out_c = t2 * a[c,c]                     (ACT, single-src)
                i, j = [d for d in range(C) if d != c]
                if abs(a[c, i]) > abs(a[c, j]):
                    i, j = j, i
                sl = lambda d: slice(d * F, (d + 1) * F)
                t1 = t_pool.tile([P, F], f32)
                nc.vector.scalar_tensor_tensor(
                    t1[:], it[:, sl(i)], float(a[c, i] / a[c, j]), it[:, sl(j)],
                    mybir.AluOpType.mult, mybir.AluOpType.add,
                )
                t2 = t_pool.tile([P, F], f32, tag="t2")
                nc.vector.scalar_tensor_tensor(
                    t2[:], t1[:], float(a[c, j] / a[c, c]), it[:, sl(c)],
                    mybir.AluOpType.mult, mybir.AluOpType.add,
                )
                nc.scalar.mul(ot[:, sl(c)], t2[:], float(a[c, c]))
            nc.scalar.dma_start(yr[b], ot[:].rearrange("p (c f) -> p c f", c=C))

    nc.compile()
    return nc


_NC = None


def _get_nc():
    global _NC
    if _NC is None:
        _NC = build_nc()
    return _NC


def _in_maps(i_en):
    xs = np.ascontiguousarray(np.asarray(i_en, dtype=np.float32)).reshape(
        N_CORES, B_PER_CORE, C, H, W
    )
    return [{"x": xs[i]} for i in range(N_CORES)]


def kernel(i_co=None, i_en=None, **_):
    res = run_bass_kernel_spmd(_get_nc(), _in_maps(i_en), list(range(N_CORES)))
    return np.concatenate(
        [res.results[i]["y"] for i in range(N_CORES)], axis=0
    )


# revision 7
# speedup vs baseline: 1.2858x; 1.0745x over previous
"""JPEGBase (nn_JPEGBase_240518169043) Trainium2 kernel.

The reference computes rgb->yuv, *255, blockwise 8x8 DCT, blockwise IDCT
(compress() is identity), /255, yuv->rgb.  The orthonormal DCT/IDCT pair and
the *255 / /255 cancel exactly, so the remaining math is a per-pixel 3x3
color-matrix roundtrip A = yuv2rgb @ rgb2yuv applied along the channel dim
(float32 discrepancy vs. the reference's explicit DCT roundtrip is ~1.5e-7
relative).  i_co is unused by the reference.

Sharding: pure data parallelism - batch 32 -> 4 images per core across 8
cores.  Per core the kernel streams 4 images of [3,512,512] f32 through SBUF
([128,2048] per plane), computes the three output planes as weighted sums of
the three input planes (2 DVE scalar_tensor_tensor ops + 1 ACT scale per
output plane), and streams them back.  Memory-bound: ~25 MB of HBM traffic
per core.
"""

import numpy as np
from contextlib import ExitStack

import concourse.bass as bass  # noqa: F401  (engine namespaces live on nc)
import concourse.tile as tile
from concourse import bacc, mybir
from concourse.bass_utils import run_bass_kernel_spmd

N_CORES = 8
B_FULL = 32
B_PER_CORE = B_FULL // N_CORES  # 4
C = 3
H = 512
W = 512
P = 128               # SBUF partitions
F = (H * W) // P      # 2048 floats per partition per plane


def _color_matrix():
    # kornia rgb_to_yuv / yuv_to_rgb coefficient matrices, composed in f64.
    m = np.array(
        [[0.299, 0.587, 0.114],
         [-0.147, -0.289, 0.436],
         [0.615, -0.515, -0.100]], dtype=np.float64)
    n = np.array(
        [[1.0, 0.0, 1.14],
         [1.0, -0.396, -0.581],
         [1.0, 2.029, 0.0]], dtype=np.float64)
    return n @ m


def build_nc():
    """Build + compile the per-core Bass program (same program on all cores)."""
    a = _color_matrix()
    nc = bacc.Bacc(
        "TRN2", target_bir_lowering=False, debug=False, num_devices=N_CORES
    )
    x = nc.dram_tensor(
        "x", [B_PER_CORE, C, H, W], mybir.dt.float32, kind="ExternalInput"
    ).ap()
    y = nc.dram_tensor(
        "y", [B_PER_CORE, C, H, W], mybir.dt.float32, kind="ExternalOutput"
    ).ap()
    # [b, 128, c, 2048]; partition p covers image rows [4p, 4p+4) (contiguous);
    # dim order matches the SBUF tile view [p, c, f].
    xr = x.rearrange("b c (hp hs) w -> b hp c (hs w)", hp=P)
    yr = y.rearrange("b c (hp hs) w -> b hp c (hs w)", hp=P)

    f32 = mybir.dt.float32
    HALVES = 2                  # groups per image
    F2 = F // HALVES            # free elems per plane per group
    with tile.TileContext(nc) as tc, ExitStack() as ctx:
        in_pool = ctx.enter_context(tc.tile_pool(name="in", bufs=5))
        out_pool = ctx.enter_context(tc.tile_pool(name="out", bufs=4))
        t_pool = ctx.enter_context(tc.tile_pool(name="tmp", bufs=4))

        for g in range(B_PER_CORE * HALVES):
            b, h = divmod(g, HALVES)
            fsl = slice(h * F2, (h + 1) * F2)
            # 1.5 MB transfers, half an image each.  Loads on the SP HWDGE
            # ring, stores on the ACT ring: each ring is FIFO per issuing
            # engine, so stores waiting on compute must not block loads.
            # ACT computes the *final* op per plane, so its store push never
            # waits on another engine.
            it = in_pool.tile([P, C * F2], f32)
            nc.sync.dma_start(
                it[:].rearrange("p (c f) -> p c f", c=C), xr[b][:, :, fsl]
            )
            ot = out_pool.tile([P, C * F2], f32)
            for c in range(C):
                # out_c = a[c,i]*X_i + a[c,j]*X_j + a[c,c]*X_c, diagonal term
                # largest; (i, j) = off-diagonals with |a_i| <= |a_j|:
                #   t1    = X_i * (a[c,i]/a[c,j]) + X_j     (DVE stt)
                #   t2    = t1 * (a[c,j]/a[c,c]) + X_c      (DVE stt)
                #   out_c = t2 * a[c,c]                     (ACT, single-src)
                i, j = [d for d in range(C) if d != c]
                if abs(a[c, i]) > abs(a[c, j]):
                    i, j = j, i
                sl = lambda d: slice(d * F2, (d + 1) * F2)
                t1 = t_pool.tile([P, F2], f32)
                nc.vector.scalar_tensor_tensor(
                    t1[:], it[:, sl(i)], float(a[c, i] / a[c, j]), it[:, sl(j)],
                    mybir.AluOpType.mult, mybir.AluOpType.add,
                )
                t2 = t_pool.tile([P, F2], f32, tag="t2")
                nc.vector.scalar_tensor_tensor(
                    t2[:], t1[:], float(a[c, j] / a[c, c]), it[:, sl(c)],
                    mybir.AluOpType.mult, mybir.AluOpType.add,
                )
                nc.scalar.mul(ot[:, sl(c)], t2[:], float(a[c, c]))
            nc.scalar.dma_start(
                yr[b][:, :, fsl], ot[:].rearrange("p (c f) -> p c f", c=C)
            )

    nc.compile()
    return nc


_NC = None


def _get_nc():
    global _NC
    if _NC is None:
        _NC = build_nc()
    return _NC


def _in_maps(i_en):
    xs = np.ascontiguousarray(np.asarray(i_en, dtype=np.float32)).reshape(
        N_CORES, B_PER_CORE, C, H, W
    )
    return [{"x": xs[i]} for i in range(N_CORES)]


def kernel(i_co=None, i_en=None, **_):
    res = run_bass_kernel_spmd(_get_nc(), _in_maps(i_en), list(range(N_CORES)))
    return np.concatenate(
        [res.results[i]["y"] for i in range(N_CORES)], axis=0
    )
